# revision 16
# baseline (speedup 1.0000x reference)
"""BatchTopK tied SAE kernel for 8 Trainium2 NeuronCores.

Data-parallel over the 4096 tokens: each core takes 512 rows of x, full W.
Per core:
  encode : pre' = x @ W + (b_enc + tb), via 3-term fp16 split matmul
           (xh*wh + xh*wl + xl*wh, W pre-scaled by 2^10 to dodge fp16
           denormals) -> s' = relu(pre') ; streaming per-row top-64 via
           DVE max8 + match_replace with a running-64 merge
  mask   : f = (s' >= t_row) * (s' - tb)   (t_row = 64th largest of s')
  decode : recon^T = W^T @ f^T + b_dec, fp16 matmul; W^T produced by
           fp16 DMA-xbar transposes of the encode-side wh tiles.
"""
import numpy as np

import concourse.bass as bass
import concourse.tile as tile
import concourse.mybir as mybir
from concourse.vector_clock import ScopedClock
from concourse import bass2jax

F32 = mybir.dt.float32
F16 = mybir.dt.float16

N_CORES = 8
N_FULL = 4096
R = N_FULL // N_CORES          # 512 rows per core
RT = R // 128                  # 4 row tiles
D = 8192                       # d_in
H = 16384                      # d_hidden
TOPK = 64
KC = D // 128                  # 64 k-chunks
NGW = 1024                     # encode hidden-col group width
NG = H // NGW                  # 16 groups
NT = NGW // 512                # 2 n-tiles per group
WS = 1024.0                    # fp16 W scale
NEG = -1e30

# decode tiling
DB_SIZES = [6] * 10 + [4]      # 11 batches of d-tiles (64 total)
assert sum(DB_SIZES) == KC
HS = H // 1024                 # 16 h-supers of 1024


class PatchedTC(tile.TileContext):
    """Tail drain in this walrus build only accepts one sync-wait; hoist
    the waits onto single-wait SP NOPs."""

    def _drain_and_barrier(self, tick_clock, wait_clock):
        nop0 = self.nc.sync.nop()
        wait_clock.add_sem_waits(nop0.ins, ScopedClock({None: tick_clock.global_clock}))
        si = nop0.ins.sync_info
        waits = list(si.on_wait or []) if si else []
        if si and len(waits) > 1:
            si.on_wait = waits[:1]
            for w in waits[1:]:
                nop = self.nc.sync.nop()
                nop.ins.sync_info = mybir.SyncInfo(on_wait=[w], on_update=[])
        self.nc.sync.drain()
        self.nc.all_engine_barrier()
        popped = self.nc._tile_sem_poison_stack.pop()
        assert popped is self._sem_poison
        self.nc.clear_and_free_semaphores(list(self.sems.allocated().values()))
        self.nc.all_engine_barrier()


def split_waits(nc, limit=1):
    """Walrus in this image rejects instructions with more than ~1 sync
    wait; hoist excess waits onto same-engine NOPs placed just before."""
    n_added = 0
    for func in nc.m.functions:
        for blk in func.blocks:
            insts = list(blk.instructions)
            out = []
            changed = False
            for ins in insts:
                si = ins.sync_info
                waits = list(si.on_wait or []) if si else []
                if len(waits) > limit:
                    keep = waits[-limit:]
                    for w in waits[:-limit]:
                        nop = mybir.InstNoOp(
                            name=f"nopw-{nc.next_id()}",
                            ins=[], outs=[],
                            engine=ins.engine,
                            sync_info=mybir.SyncInfo(on_wait=[w], on_update=[]),
                        )
                        nc.register_instruction(nop)
                        out.append(nop)
                        n_added += 1
                    si.on_wait = keep
                    changed = True
                out.append(ins)
            if changed:
                blk.instructions = out
    return n_added


def dedup_ldweights(nc):
    """Drop an InstLdweights identical to the previous one when only
    InstMatmult instructions sit between them on the PE stream (walrus
    ldw-opt is disabled in this image)."""
    import concourse.mybir as _mb
    n_drop = 0
    for func in nc.m.functions:
        for blk in func.blocks:
            last_sig = None
            out = []
            for ins in blk.instructions:
                ty = type(ins).__name__
                if ty == "InstLdweights":
                    sig = repr(ins.ins) + repr(getattr(ins, "is_transpose", None))
                    has_sync = bool(ins.sync_info and
                                    (ins.sync_info.on_wait or ins.sync_info.on_update))
                    if sig == last_sig and not has_sync:
                        n_drop += 1
                        continue
                    last_sig = sig
                elif ty == "InstMatmult":
                    # transpose-mode and fp32 (self-loading) matmuls clobber
                    # the loaded weight state
                    wdt = None
                    try:
                        wdt = ins.ins[1].dtype
                    except Exception:
                        pass
                    if getattr(ins, "is_transpose", False) or wdt == mybir.dt.float32:
                        last_sig = None
                elif ty in ("InstNoOp", "InstDMACopy", "InstDmaTransposeAnt",
                            "InstActivation", "InstTensorScalarPtr",
                            "InstTensorCopy", "InstTensorTensor", "InstMax",
                            "InstMatchReplace", "InstMemset",
                            "InstEventSemaphore", "InstTensorReduce"):
                    pass  # non-PE instructions do not touch PE weights
                else:
                    last_sig = None
                out.append(ins)
            if n_drop:
                blk.instructions = out
    return n_drop


def build_program(phases=("prep", "enc", "mask", "dec")):
    nc = bass.Bass("TRN2", target_bir_lowering=False, debug=False)

    x_d = nc.dram_tensor("x", [R, D], F32, kind="ExternalInput").ap()
    w_d = nc.dram_tensor("W", [D, H], F32, kind="ExternalInput").ap()
    benc_d = nc.dram_tensor("b_enc", [H], F32, kind="ExternalInput").ap()
    bdec_d = nc.dram_tensor("b_dec", [D], F32, kind="ExternalInput").ap()
    tb_d = nc.dram_tensor("tb", [H], F32, kind="ExternalInput").ap()
    id_d = nc.dram_tensor("ident", [128, 128], F32, kind="ExternalInput").ap()

    recon_d = nc.dram_tensor("recon", [R, D], F32, kind="ExternalOutput").ap()
    f_d = nc.dram_tensor("f", [R, H], F32, kind="ExternalOutput").ap()

    # DRAM scratch
    s_buf = nc.dram_tensor("s_buf", [R, H], F32).ap()
    ft16_d = nc.dram_tensor("ft16", [H, R], F16).ap()
    # W^T fp16, tiled: [kc, h_lo, hcg, d_lo] -- 2KB contiguous runs for both
    # the encode-side writes (fixed kc, 8 consecutive hcg) and the decode-side
    # reads (fixed kc+h_lo, 8 consecutive hcg).
    wt16_d = nc.dram_tensor("wt16", [KC, 128, H // 128, 128], F16).ap()
    bts_d = nc.dram_tensor("bts", [H], F32).ap()

    with PatchedTC(nc) as tc:
        with tc.tile_pool(name="persist", bufs=1) as persist:

            ones_t = persist.tile([1, 128], F32)
            nc.vector.memset(ones_t[:], 1.0)
            id_t = persist.tile([128, 128], F32)
            nc.sync.dma_start(id_t[:], id_d[:])
            bdec_t = persist.tile([128, KC], F32)
            nc.sync.dma_start(bdec_t[:], bdec_d.rearrange("(t p) -> p t", p=128))
            run64 = persist.tile([128, RT * 64], F32)
            nc.vector.memset(run64[:], NEG)

            # ---------------- prep ----------------
            xpool_cm = tc.tile_pool(name="xops", bufs=1)
            xops = xpool_cm.__enter__()
            # fp16 x operands, all row tiles resident: [rt, kc] chunks of [128,128]
            xh_all = xops.tile([128, RT * KC * 128], F16)
            xl_all = xops.tile([128, RT * KC * 128], F16)
            if "prep" in phases:
                with tc.tile_pool(name="prep", bufs=2) as prep, \
                     tc.tile_pool(name="ppsum", bufs=2, space="PSUM") as pps:
                    BW = 1024
                    for i in range(H // BW):
                        bsl = slice(i * BW, (i + 1) * BW)
                        b1 = prep.tile([1, BW], F32)
                        nc.sync.dma_start(b1[:], benc_d[None, bsl])
                        b2 = prep.tile([1, BW], F32)
                        nc.sync.dma_start(b2[:], tb_d[None, bsl])
                        b3 = prep.tile([1, BW], F32)
                        nc.vector.tensor_add(b3[:], b1[:], b2[:])
                        nc.vector.tensor_scalar_mul(b3[:], b3[:], WS)
                        nc.sync.dma_start(bts_d[None, bsl], b3[:])

                    for rt in range(RT):
                        xrow = prep.tile([128, D], F32, bufs=1)
                        nc.sync.dma_start(xrow[:], x_d[rt * 128:(rt + 1) * 128, :])
                        for g in range(KC // 4):  # 4 transposes per psum bank
                            pst = pps.tile([128, 512], F32)
                            for j in range(4):
                                kc = g * 4 + j
                                nc.tensor.transpose(
                                    pst[:, j * 128:(j + 1) * 128],
                                    xrow[:, kc * 128:(kc + 1) * 128], id_t[:])
                            x32 = prep.tile([128, 512], F32)
                            nc.scalar.copy(x32[:], pst[:])
                            base = (rt * KC + g * 4) * 128
                            sl = slice(base, base + 512)
                            nc.scalar.copy(xh_all[:, sl], x32[:])
                            nc.vector.tensor_sub(xl_all[:, sl], x32[:], xh_all[:, sl])

            # ---------------- encode ----------------
            if "enc" in phases:
                with tc.tile_pool(name="enc", bufs=2) as enc, \
                     tc.tile_pool(name="encw", bufs=3) as encw, \
                     tc.tile_pool(name="encs", bufs=2) as encs, \
                     tc.tile_pool(name="epsum", bufs=1, space="PSUM") as pps:
                    for ng in range(NG):
                        csl = slice(ng * NGW, (ng + 1) * NGW)
                        bts_t = enc.tile([1, NGW], F32)
                        nc.sync.dma_start(bts_t[:], bts_d[None, csl])

                        ps = [pps.tile([128, 512], F32, tag=f"eps{i}",
                                       name=f"eps{i}_{ng}")
                              for i in range(RT * NT)]

                        for kc in range(KC):
                            wch = encw.tile([128, NGW], F32, tag="wch")
                            nc.sync.dma_start(
                                wch[:], w_d[kc * 128:(kc + 1) * 128, csl])
                            wh = encw.tile([128, NGW], F16, tag="wh")
                            nc.scalar.mul(wh[:], wch[:], WS)
                            wl = encw.tile([128, NGW], F16, tag="wl")
                            nc.vector.scalar_tensor_tensor(
                                wl[:], wch[:], WS, wh[:],
                                mybir.AluOpType.mult, mybir.AluOpType.subtract)

                            for rt in range(RT):
                                xsl = slice((rt * KC + kc) * 128,
                                            (rt * KC + kc) * 128 + 128)
                                for nt in range(NT):
                                    nc.tensor.matmul(
                                        ps[rt * NT + nt][:], xh_all[:, xsl],
                                        wh[:, nt * 512:(nt + 1) * 512],
                                        start=(kc == 0), stop=False)
                                for nt in range(NT):
                                    nc.tensor.matmul(
                                        ps[rt * NT + nt][:], xh_all[:, xsl],
                                        wl[:, nt * 512:(nt + 1) * 512],
                                        start=False, stop=False)
                                for nt in range(NT):
                                    nc.tensor.matmul(
                                        ps[rt * NT + nt][:], xl_all[:, xsl],
                                        wh[:, nt * 512:(nt + 1) * 512],
                                        start=False, stop=False)

                            # write W^T fp16 tiles for decode
                            xbt = encw.tile([128, 8, 128], F16, tag="xbt")
                            nc.sync.dma_start_transpose(xbt[:], wh[:])
                            nc.sync.dma_start(
                                wt16_d[kc, :, ng * 8:(ng + 1) * 8, :], xbt[:])

                        for rt in range(RT):
                            for nt in range(NT):
                                nc.tensor.matmul(
                                    ps[rt * NT + nt][:], ones_t[:],
                                    bts_t[:, nt * 512:(nt + 1) * 512],
                                    start=False, stop=True)

                        for rt in range(RT):
                            st = encs.tile([128, NGW], F32, tag=f"st{rt}")
                            for nt in range(NT):
                                nc.scalar.activation(
                                    st[:, nt * 512:(nt + 1) * 512],
                                    ps[rt * NT + nt][:],
                                    mybir.ActivationFunctionType.Relu,
                                    bias=0.0, scale=1.0 / WS)
                            nc.sync.dma_start(
                                s_buf[rt * 128:(rt + 1) * 128, csl], st[:])

                            cand = enc.tile([128, 64], F32, tag=f"cand{rt}")
                            for r in range(8):
                                nc.vector.max(cand[:, r * 8:(r + 1) * 8], st[:])
                                nc.vector.match_replace(
                                    st[:], cand[:, r * 8:(r + 1) * 8], st[:], NEG)
                            mb = enc.tile([128, 128], F32, tag=f"mb{rt}")
                            rsl = slice(rt * 64, (rt + 1) * 64)
                            nc.vector.tensor_copy(mb[:, 0:64], run64[:, rsl])
                            nc.vector.tensor_copy(mb[:, 64:128], cand[:])
                            for r in range(8):
                                nc.vector.max(
                                    run64[:, rt * 64 + r * 8: rt * 64 + r * 8 + 8],
                                    mb[:])
                                nc.vector.match_replace(
                                    mb[:], run64[:, rt * 64 + r * 8: rt * 64 + r * 8 + 8],
                                    mb[:], NEG)

            xpool_cm.__exit__(None, None, None)

            # ---------------- mask ----------------
            if "mask" in phases:
                with tc.tile_pool(name="mask", bufs=1) as maskp, \
                     tc.tile_pool(name="maskw", bufs=3) as maskw:
                    tbb = maskp.tile([128, H], F32)
                    for p in range(128):
                        nc.sync.dma_start(tbb[p:p + 1, :], tb_d[None, :])
                    for ch in range(NG):
                        csl = slice(ch * NGW, (ch + 1) * NGW)
                        for rt in range(RT):
                            sre = maskw.tile([128, NGW], F32, tag="sre")
                            nc.sync.dma_start(
                                sre[:], s_buf[rt * 128:(rt + 1) * 128, csl])
                            sub = maskw.tile([128, NGW], F32, tag="sub")
                            nc.vector.tensor_sub(sub[:], sre[:], tbb[:, csl])
                            fch = maskw.tile([128, NGW], F32, tag="fch")
                            nc.vector.scalar_tensor_tensor(
                                fch[:], sre[:],
                                run64[:, rt * 64 + 63: rt * 64 + 64], sub[:],
                                mybir.AluOpType.is_ge, mybir.AluOpType.mult)
                            nc.sync.dma_start(
                                f_d[rt * 128:(rt + 1) * 128, csl], fch[:])
                            f16c = maskw.tile([128, NGW], F16, tag="f16c")
                            nc.vector.tensor_copy(f16c[:], fch[:])
                            fxb = maskw.tile([128, 8, 128], F16, tag="fxb")
                            nc.sync.dma_start_transpose(fxb[:], f16c[:])
                            nc.sync.dma_start(
                                ft16_d[ch * NGW:(ch + 1) * NGW,
                                       rt * 128:(rt + 1) * 128]
                                .rearrange("(c p) m -> p c m", p=128), fxb[:])

            # ---------------- decode ----------------
            if "dec" in phases:
                with tc.tile_pool(name="dec", bufs=3) as dec, \
                     tc.tile_pool(name="dect", bufs=2) as dect, \
                     tc.tile_pool(name="dpsum", bufs=1, space="PSUM") as pps:
                    db0 = 0
                    for db_n in DB_SIZES:
                        dsl = slice(db0, db0 + db_n)
                        psd = [pps.tile([128, 512], F32, tag=f"dps{j}",
                                        name=f"dps{j}_{db0}")
                               for j in range(db_n)]
                        for hs in range(HS):
                            ftc = dec.tile([128, 8, 512], F16, tag="ftc")
                            nc.sync.dma_start(
                                ftc[:],
                                ft16_d[hs * 1024:(hs + 1) * 1024, :]
                                .rearrange("(c p) m -> p c m", p=128))
                            wts = dec.tile([128, db_n, 8, 128], F16, tag="wts")
                            nc.sync.dma_start(
                                wts[:],
                                wt16_d[dsl, :, hs * 8:(hs + 1) * 8, :]
                                .rearrange("t p c m -> p t c m"))
                            for hc in range(8):
                                for j in range(db_n):
                                    nc.tensor.matmul(
                                        psd[j][:], wts[:, j, hc, :],
                                        ftc[:, hc, :],
                                        start=(hs == 0 and hc == 0),
                                        stop=(hs == HS - 1 and hc == 7))
                        for j in range(db_n):
                            dt = db0 + j
                            rT = dect.tile([128, 512], F32, tag="rT")
                            nc.scalar.activation(
                                rT[:], psd[j][:],
                                mybir.ActivationFunctionType.Identity,
                                bias=bdec_t[:, dt:dt + 1], scale=1.0 / WS)
                            pst = pps.tile([128, 512], F32, tag="dtr", bufs=2)
                            for mb in range(4):
                                nc.tensor.transpose(
                                    pst[:, mb * 128:(mb + 1) * 128],
                                    rT[:, mb * 128:(mb + 1) * 128], id_t[:])
                            rc = dect.tile([128, 512], F32, tag="rc")
                            nc.scalar.copy(rc[:], pst[:])
                            for mb in range(4):
                                nc.sync.dma_start(
                                    recon_d[mb * 128:(mb + 1) * 128,
                                            dt * 128:(dt + 1) * 128],
                                    rc[:, mb * 128:(mb + 1) * 128])
                        db0 += db_n

    n_dropped = dedup_ldweights(nc)
    split_waits(nc)
    return nc


_NC_CACHE = {}


def _get_program():
    key = "full"
    if key not in _NC_CACHE:
        _NC_CACHE[key] = build_program()
    return _NC_CACHE[key]


def kernel(x, W, b_enc, b_dec, tiebreaker):
    x = np.ascontiguousarray(np.asarray(x, dtype=np.float32))
    W = np.ascontiguousarray(np.asarray(W, dtype=np.float32))
    b_enc = np.ascontiguousarray(np.asarray(b_enc, dtype=np.float32))
    b_dec = np.ascontiguousarray(np.asarray(b_dec, dtype=np.float32))
    tb = np.ascontiguousarray(np.asarray(tiebreaker, dtype=np.float32))
    ident = np.eye(128, dtype=np.float32)

    nc = _get_program()
    in_maps = []
    for c in range(N_CORES):
        in_maps.append({
            "x": x[c * R:(c + 1) * R, :],
            "W": W,
            "b_enc": b_enc,
            "b_dec": b_dec,
            "tb": tb,
            "ident": ident,
        })
    results = bass2jax.run_bass_via_pjrt(nc, in_maps, n_cores=N_CORES)
    recon = np.concatenate([results[c]["recon"] for c in range(N_CORES)], axis=0)
    f = np.concatenate([results[c]["f"] for c in range(N_CORES)], axis=0)
    return recon, f


# revision 19
# speedup vs baseline: 1.0628x; 1.0628x over previous
"""BatchTopK tied SAE kernel for 8 Trainium2 NeuronCores.

Data-parallel over the 4096 tokens: each core takes 512 rows of x, full W.
Per core:
  encode : pre' = x @ W + (b_enc + tb), via 3-term fp16 split matmul
           (xh*wh + xh*wl + xl*wh, W pre-scaled by 2^10 to dodge fp16
           denormals) -> s' = relu(pre') ; streaming per-row top-64 via
           DVE max8 + match_replace with a running-64 merge
  mask   : f = (s' >= t_row) * (s' - tb)   (t_row = 64th largest of s')
  decode : recon^T = W^T @ f^T + b_dec, fp16 matmul; W^T produced by
           fp16 DMA-xbar transposes of the encode-side wh tiles.
"""
import numpy as np

import concourse.bass as bass
import concourse.tile as tile
import concourse.mybir as mybir
from concourse.vector_clock import ScopedClock
from concourse import bass2jax

F32 = mybir.dt.float32
F16 = mybir.dt.float16

N_CORES = 8
N_FULL = 4096
R = N_FULL // N_CORES          # 512 rows per core
RT = R // 128                  # 4 row tiles
D = 8192                       # d_in
H = 16384                      # d_hidden
TOPK = 64
KC = D // 128                  # 64 k-chunks
NGW = 1024                     # encode hidden-col group width
NG = H // NGW                  # 16 groups
NT = NGW // 512                # 2 n-tiles per group
WS = 1024.0                    # fp16 W scale
NEG = -1e30

# decode tiling
DB_SIZES = [6] * 10 + [4]      # 11 batches of d-tiles (64 total)
assert sum(DB_SIZES) == KC
HS = H // 1024                 # 16 h-supers of 1024


class PatchedTC(tile.TileContext):
    """Tail drain in this walrus build only accepts one sync-wait; hoist
    the waits onto single-wait SP NOPs."""

    def _drain_and_barrier(self, tick_clock, wait_clock):
        nop0 = self.nc.sync.nop()
        wait_clock.add_sem_waits(nop0.ins, ScopedClock({None: tick_clock.global_clock}))
        si = nop0.ins.sync_info
        waits = list(si.on_wait or []) if si else []
        if si and len(waits) > 1:
            si.on_wait = waits[:1]
            for w in waits[1:]:
                nop = self.nc.sync.nop()
                nop.ins.sync_info = mybir.SyncInfo(on_wait=[w], on_update=[])
        self.nc.sync.drain()
        self.nc.all_engine_barrier()
        popped = self.nc._tile_sem_poison_stack.pop()
        assert popped is self._sem_poison
        self.nc.clear_and_free_semaphores(list(self.sems.allocated().values()))
        self.nc.all_engine_barrier()


def split_waits(nc, limit=1):
    """Walrus in this image rejects instructions with more than ~1 sync
    wait; hoist excess waits onto same-engine NOPs placed just before."""
    n_added = 0
    for func in nc.m.functions:
        for blk in func.blocks:
            insts = list(blk.instructions)
            out = []
            changed = False
            for ins in insts:
                si = ins.sync_info
                waits = list(si.on_wait or []) if si else []
                if len(waits) > limit:
                    keep = waits[-limit:]
                    for w in waits[:-limit]:
                        nop = mybir.InstNoOp(
                            name=f"nopw-{nc.next_id()}",
                            ins=[], outs=[],
                            engine=ins.engine,
                            sync_info=mybir.SyncInfo(on_wait=[w], on_update=[]),
                        )
                        nc.register_instruction(nop)
                        out.append(nop)
                        n_added += 1
                    si.on_wait = keep
                    changed = True
                out.append(ins)
            if changed:
                blk.instructions = out
    return n_added


def dedup_ldweights(nc):
    """Drop an InstLdweights identical to the previous one when only
    InstMatmult instructions sit between them on the PE stream (walrus
    ldw-opt is disabled in this image)."""
    import concourse.mybir as _mb
    n_drop = 0
    for func in nc.m.functions:
        for blk in func.blocks:
            last_sig = None
            out = []
            for ins in blk.instructions:
                ty = type(ins).__name__
                if ty == "InstLdweights":
                    sig = repr(ins.ins) + repr(getattr(ins, "is_transpose", None))
                    has_sync = bool(ins.sync_info and
                                    (ins.sync_info.on_wait or ins.sync_info.on_update))
                    if sig == last_sig and not has_sync:
                        n_drop += 1
                        continue
                    last_sig = sig
                elif ty == "InstMatmult":
                    # transpose-mode and fp32 (self-loading) matmuls clobber
                    # the loaded weight state
                    wdt = None
                    try:
                        wdt = ins.ins[1].dtype
                    except Exception:
                        pass
                    if getattr(ins, "is_transpose", False) or wdt == mybir.dt.float32:
                        last_sig = None
                elif ty in ("InstNoOp", "InstDMACopy", "InstDmaTransposeAnt",
                            "InstActivation", "InstTensorScalarPtr",
                            "InstTensorCopy", "InstTensorTensor", "InstMax",
                            "InstMatchReplace", "InstMemset",
                            "InstEventSemaphore", "InstTensorReduce"):
                    pass  # non-PE instructions do not touch PE weights
                else:
                    last_sig = None
                out.append(ins)
            if n_drop:
                blk.instructions = out
    return n_drop


def build_program(phases=("prep", "enc", "mask", "dec")):
    nc = bass.Bass("TRN2", target_bir_lowering=False, debug=False)

    x_d = nc.dram_tensor("x", [R, D], F32, kind="ExternalInput").ap()
    w_d = nc.dram_tensor("W", [D, H], F32, kind="ExternalInput").ap()
    benc_d = nc.dram_tensor("b_enc", [H], F32, kind="ExternalInput").ap()
    bdec_d = nc.dram_tensor("b_dec", [D], F32, kind="ExternalInput").ap()
    tb_d = nc.dram_tensor("tb", [H], F32, kind="ExternalInput").ap()
    id_d = nc.dram_tensor("ident", [128, 128], F32, kind="ExternalInput").ap()

    recon_d = nc.dram_tensor("recon", [R, D], F32, kind="ExternalOutput").ap()
    f_d = nc.dram_tensor("f", [R, H], F32, kind="ExternalOutput").ap()

    # DRAM scratch
    s_buf = nc.dram_tensor("s_buf", [R, H], F32).ap()
    ft16_d = nc.dram_tensor("ft16", [H, R], F16).ap()
    # W^T fp16, tiled: [kc, h_lo, hcg, d_lo] -- 2KB contiguous runs for both
    # the encode-side writes (fixed kc, 8 consecutive hcg) and the decode-side
    # reads (fixed kc+h_lo, 8 consecutive hcg).
    wt16_d = nc.dram_tensor("wt16", [KC, 128, H // 128, 128], F16).ap()
    bts_d = nc.dram_tensor("bts", [H], F32).ap()

    with PatchedTC(nc) as tc:
        with tc.tile_pool(name="persist", bufs=1) as persist:

            ones_t = persist.tile([1, 128], F32)
            nc.vector.memset(ones_t[:], 1.0)
            id_t = persist.tile([128, 128], F32)
            nc.sync.dma_start(id_t[:], id_d[:])
            bdec_t = persist.tile([128, KC], F32)
            nc.sync.dma_start(bdec_t[:], bdec_d.rearrange("(t p) -> p t", p=128))
            run64 = persist.tile([128, RT * 64], F32)
            nc.vector.memset(run64[:], NEG)

            # ---------------- prep ----------------
            xpool_cm = tc.tile_pool(name="xops", bufs=1)
            xops = xpool_cm.__enter__()
            # fp16 x operands, all row tiles resident: [rt, kc] chunks of [128,128]
            xh_all = xops.tile([128, RT * KC * 128], F16)
            xl_all = xops.tile([128, RT * KC * 128], F16)
            if "prep" in phases:
                with tc.tile_pool(name="prep", bufs=2) as prep, \
                     tc.tile_pool(name="ppsum", bufs=2, space="PSUM") as pps:
                    BW = 1024
                    for i in range(H // BW):
                        bsl = slice(i * BW, (i + 1) * BW)
                        b1 = prep.tile([1, BW], F32)
                        nc.sync.dma_start(b1[:], benc_d[None, bsl])
                        b2 = prep.tile([1, BW], F32)
                        nc.sync.dma_start(b2[:], tb_d[None, bsl])
                        b3 = prep.tile([1, BW], F32)
                        nc.vector.tensor_add(b3[:], b1[:], b2[:])
                        nc.vector.tensor_scalar_mul(b3[:], b3[:], WS)
                        nc.sync.dma_start(bts_d[None, bsl], b3[:])

                    for rt in range(RT):
                        xrow = prep.tile([128, D], F32, bufs=1)
                        nc.sync.dma_start(xrow[:], x_d[rt * 128:(rt + 1) * 128, :])
                        for g in range(KC // 4):  # 4 transposes per psum bank
                            pst = pps.tile([128, 512], F32)
                            for j in range(4):
                                kc = g * 4 + j
                                nc.tensor.transpose(
                                    pst[:, j * 128:(j + 1) * 128],
                                    xrow[:, kc * 128:(kc + 1) * 128], id_t[:])
                            x32 = prep.tile([128, 512], F32)
                            nc.scalar.copy(x32[:], pst[:])
                            base = (rt * KC + g * 4) * 128
                            sl = slice(base, base + 512)
                            nc.scalar.copy(xh_all[:, sl], x32[:])
                            nc.vector.tensor_sub(xl_all[:, sl], x32[:], xh_all[:, sl])

            # ---------------- encode ----------------
            if "enc" in phases:
                with tc.tile_pool(name="enc", bufs=2) as enc, \
                     tc.tile_pool(name="encw", bufs=3) as encw, \
                     tc.tile_pool(name="encs", bufs=2) as encs, \
                     tc.tile_pool(name="epsum", bufs=1, space="PSUM") as pps:
                    pending_ops = []

                    for ng in range(NG):
                        csl = slice(ng * NGW, (ng + 1) * NGW)
                        bts_t = enc.tile([1, NGW], F32)
                        nc.sync.dma_start(bts_t[:], bts_d[None, csl])

                        ps = [pps.tile([128, 512], F32, tag=f"eps{i}",
                                       name=f"eps{i}_{ng}")
                              for i in range(RT * NT)]

                        for kc in range(KC):
                            wch = encw.tile([128, NGW], F32, tag="wch")
                            nc.sync.dma_start(
                                wch[:], w_d[kc * 128:(kc + 1) * 128, csl])
                            wh = encw.tile([128, NGW], F16, tag="wh")
                            nc.scalar.mul(wh[:], wch[:], WS)
                            wl = encw.tile([128, NGW], F16, tag="wl")
                            nc.vector.scalar_tensor_tensor(
                                wl[:], wch[:], WS, wh[:],
                                mybir.AluOpType.mult, mybir.AluOpType.subtract)

                            for rt in range(RT):
                                xsl = slice((rt * KC + kc) * 128,
                                            (rt * KC + kc) * 128 + 128)
                                for nt in range(NT):
                                    nc.tensor.matmul(
                                        ps[rt * NT + nt][:], xh_all[:, xsl],
                                        wh[:, nt * 512:(nt + 1) * 512],
                                        start=(kc == 0), stop=False)
                                for nt in range(NT):
                                    nc.tensor.matmul(
                                        ps[rt * NT + nt][:], xh_all[:, xsl],
                                        wl[:, nt * 512:(nt + 1) * 512],
                                        start=False, stop=False)
                                for nt in range(NT):
                                    nc.tensor.matmul(
                                        ps[rt * NT + nt][:], xl_all[:, xsl],
                                        wh[:, nt * 512:(nt + 1) * 512],
                                        start=False, stop=False)

                            # write W^T fp16 tiles for decode
                            xbt = encw.tile([128, 8, 128], F16, tag="xbt")
                            nc.sync.dma_start_transpose(xbt[:], wh[:])
                            nc.sync.dma_start(
                                wt16_d[kc, :, ng * 8:(ng + 1) * 8, :], xbt[:])

                            # drip-feed the previous group's top-k extraction
                            # between matmul operand chains so the DVE never
                            # starves the PE of wl tiles
                            for _ in range(3):
                                if pending_ops:
                                    pending_ops.pop(0)()

                        for rt in range(RT):
                            for nt in range(NT):
                                nc.tensor.matmul(
                                    ps[rt * NT + nt][:], ones_t[:],
                                    bts_t[:, nt * 512:(nt + 1) * 512],
                                    start=False, stop=True)

                        for rt in range(RT):
                            st = encs.tile([128, NGW], F32, tag=f"st{rt}",
                                           name=f"st{rt}_{ng}")
                            for nt in range(NT):
                                nc.scalar.activation(
                                    st[:, nt * 512:(nt + 1) * 512],
                                    ps[rt * NT + nt][:],
                                    mybir.ActivationFunctionType.Relu,
                                    bias=0.0, scale=1.0 / WS)
                            nc.sync.dma_start(
                                s_buf[rt * 128:(rt + 1) * 128, csl], st[:])

                            cand = enc.tile([128, 64], F32, tag=f"cand{rt}",
                                            name=f"cand{rt}_{ng}")
                            mb = enc.tile([128, 128], F32, tag=f"mb{rt}",
                                          name=f"mb{rt}_{ng}")

                            def mk_extract(st, cand, r, rt=rt):
                                def op():
                                    nc.vector.max(cand[:, r * 8:(r + 1) * 8], st[:])
                                    nc.vector.match_replace(
                                        st[:], cand[:, r * 8:(r + 1) * 8], st[:], NEG)
                                return op

                            def mk_mergecp(st, cand, mb, rt=rt):
                                def op():
                                    rsl = slice(rt * 64, (rt + 1) * 64)
                                    nc.vector.tensor_copy(mb[:, 0:64], run64[:, rsl])
                                    nc.vector.tensor_copy(mb[:, 64:128], cand[:])
                                return op

                            def mk_merge(mb, r, rt=rt):
                                def op():
                                    o = rt * 64 + r * 8
                                    nc.vector.max(run64[:, o:o + 8], mb[:])
                                    nc.vector.match_replace(
                                        mb[:], run64[:, o:o + 8], mb[:], NEG)
                                return op

                            for r in range(8):
                                pending_ops.append(mk_extract(st, cand, r))
                            pending_ops.append(mk_mergecp(st, cand, mb))
                            for r in range(8):
                                pending_ops.append(mk_merge(mb, r))

                    # flush extraction work of the final group
                    while pending_ops:
                        pending_ops.pop(0)()

            xpool_cm.__exit__(None, None, None)


            # ---------------- mask ----------------
            if "mask" in phases:
                with tc.tile_pool(name="mask", bufs=1) as maskp, \
                     tc.tile_pool(name="maskw", bufs=3) as maskw:
                    tbb = maskp.tile([128, H], F32)
                    for p in range(128):
                        nc.sync.dma_start(tbb[p:p + 1, :], tb_d[None, :])
                    for ch in range(NG):
                        csl = slice(ch * NGW, (ch + 1) * NGW)
                        for rt in range(RT):
                            sre = maskw.tile([128, NGW], F32, tag="sre")
                            nc.sync.dma_start(
                                sre[:], s_buf[rt * 128:(rt + 1) * 128, csl])
                            sub = maskw.tile([128, NGW], F32, tag="sub")
                            nc.vector.tensor_sub(sub[:], sre[:], tbb[:, csl])
                            fch = maskw.tile([128, NGW], F32, tag="fch")
                            nc.vector.scalar_tensor_tensor(
                                fch[:], sre[:],
                                run64[:, rt * 64 + 63: rt * 64 + 64], sub[:],
                                mybir.AluOpType.is_ge, mybir.AluOpType.mult)
                            nc.sync.dma_start(
                                f_d[rt * 128:(rt + 1) * 128, csl], fch[:])
                            f16c = maskw.tile([128, NGW], F16, tag="f16c")
                            nc.vector.tensor_copy(f16c[:], fch[:])
                            fxb = maskw.tile([128, 8, 128], F16, tag="fxb")
                            nc.sync.dma_start_transpose(fxb[:], f16c[:])
                            nc.sync.dma_start(
                                ft16_d[ch * NGW:(ch + 1) * NGW,
                                       rt * 128:(rt + 1) * 128]
                                .rearrange("(c p) m -> p c m", p=128), fxb[:])

            # ---------------- decode ----------------
            if "dec" in phases:
                with tc.tile_pool(name="dec", bufs=3) as dec, \
                     tc.tile_pool(name="dect", bufs=2) as dect, \
                     tc.tile_pool(name="dpsum", bufs=1, space="PSUM") as pps:
                    db0 = 0
                    for db_n in DB_SIZES:
                        dsl = slice(db0, db0 + db_n)
                        psd = [pps.tile([128, 512], F32, tag=f"dps{j}",
                                        name=f"dps{j}_{db0}")
                               for j in range(db_n)]
                        for hs in range(HS):
                            ftc = dec.tile([128, 8, 512], F16, tag="ftc")
                            nc.sync.dma_start(
                                ftc[:],
                                ft16_d[hs * 1024:(hs + 1) * 1024, :]
                                .rearrange("(c p) m -> p c m", p=128))
                            wts = dec.tile([128, db_n, 8, 128], F16, tag="wts")
                            nc.sync.dma_start(
                                wts[:],
                                wt16_d[dsl, :, hs * 8:(hs + 1) * 8, :]
                                .rearrange("t p c m -> p t c m"))
                            for hc in range(8):
                                for j in range(db_n):
                                    nc.tensor.matmul(
                                        psd[j][:], wts[:, j, hc, :],
                                        ftc[:, hc, :],
                                        start=(hs == 0 and hc == 0),
                                        stop=(hs == HS - 1 and hc == 7))
                        for j in range(db_n):
                            dt = db0 + j
                            rT = dect.tile([128, 512], F32, tag="rT")
                            nc.scalar.activation(
                                rT[:], psd[j][:],
                                mybir.ActivationFunctionType.Identity,
                                bias=bdec_t[:, dt:dt + 1], scale=1.0 / WS)
                            pst = pps.tile([128, 512], F32, tag="dtr", bufs=2)
                            for mb in range(4):
                                nc.tensor.transpose(
                                    pst[:, mb * 128:(mb + 1) * 128],
                                    rT[:, mb * 128:(mb + 1) * 128], id_t[:])
                            rc = dect.tile([128, 512], F32, tag="rc")
                            nc.scalar.copy(rc[:], pst[:])
                            for mb in range(4):
                                nc.sync.dma_start(
                                    recon_d[mb * 128:(mb + 1) * 128,
                                            dt * 128:(dt + 1) * 128],
                                    rc[:, mb * 128:(mb + 1) * 128])
                        db0 += db_n

    n_dropped = dedup_ldweights(nc)
    split_waits(nc)
    return nc


_NC_CACHE = {}


def _get_program():
    key = "full"
    if key not in _NC_CACHE:
        _NC_CACHE[key] = build_program()
    return _NC_CACHE[key]


def kernel(x, W, b_enc, b_dec, tiebreaker):
    x = np.ascontiguousarray(np.asarray(x, dtype=np.float32))
    W = np.ascontiguousarray(np.asarray(W, dtype=np.float32))
    b_enc = np.ascontiguousarray(np.asarray(b_enc, dtype=np.float32))
    b_dec = np.ascontiguousarray(np.asarray(b_dec, dtype=np.float32))
    tb = np.ascontiguousarray(np.asarray(tiebreaker, dtype=np.float32))
    ident = np.eye(128, dtype=np.float32)

    nc = _get_program()
    in_maps = []
    for c in range(N_CORES):
        in_maps.append({
            "x": x[c * R:(c + 1) * R, :],
            "W": W,
            "b_enc": b_enc,
            "b_dec": b_dec,
            "tb": tb,
            "ident": ident,
        })
    results = bass2jax.run_bass_via_pjrt(nc, in_maps, n_cores=N_CORES)
    recon = np.concatenate([results[c]["recon"] for c in range(N_CORES)], axis=0)
    f = np.concatenate([results[c]["f"] for c in range(N_CORES)], axis=0)
    return recon, f
